# revision 23
# baseline (speedup 1.0000x reference)
"""Trainium2 Bass kernel for nn_AttentionFilter (B=2,C=128,H=256,W=510).

Sharding: 8 cores = 2 batches x 4 channel-groups of 32. Per core:
  Phase A: 1x1 conv as fp8 matmul (x16-scaled fp8 weights, 1/16+bias fused
    into the PSUM->SBUF copy), spill y as fp8 to DRAM padded to W=512.
  Phase B (software-pipelined, backend of channel c-1 interleaved with
    frontend of channel c): xbar DMA-transpose of fp8 y viewed as uint16
    pairs -> pair-interleaved [w/2, 2, h] stationaries; rfft_w and fft_h as
    fp8 DoubleRow matmuls (x16-scaled fp8 DFT constants, rescale fused into
    PSUM copies); complex filter multiply as fp16 2x-mode tensor_tensor;
    variance via bn_stats + integer-magic rsqrt on Pool; freq attention
    matmul fp16 into a single PSUM bank; tanh on ACT (sigmoid = 0.5 +
    0.5tanh, the 0.5-DC correction folded into xres on host) with fused
    rstd/2 row scale writing fp8; ifft_i and irfft_j as fp8 DoubleRow
    matmuls; residual add fused with 1/16 rescale; LN stats: S1/S2 fp32
    accumulated on Pool, squares via ACT (x1/256 to bound range).
  Phase C: fp32 AllReduce of LN stats within each batch group, rsqrt via
    integer magic + 2 Newton steps, per-channel normalize, fp16 output.
"""
import sys

sys.path.insert(0, "/opt/trn_rl_repo")

import numpy as np
import ml_dtypes

import concourse.bass as bass
import concourse.mybir as mybir
import concourse.tile as tile
from concourse import bacc
from concourse.bass_utils import run_bass_kernel_spmd

B, C, H, W = 2, 128, 256, 510
WF = 256
W2 = 512
NCH = 32  # channels per core
N_CORES = 8
F32 = mybir.dt.float32
F16 = mybir.dt.float16
F8 = mybir.dt.float8e4
U16 = mybir.dt.uint16
I32 = mybir.dt.int32
AF = mybir.ActivationFunctionType
OP = mybir.AluOpType
DR = mybir.MatmulPerfMode.DoubleRow

E4M3 = ml_dtypes.float8_e4m3
CS = 16.0  # fp8 constant scale
RSQRT_MAGIC = 0x5F3759DF
INV_SQRT_2PI = float(1.0 / np.sqrt(2.0 * np.pi))

_CORR = None


def _corr_w():
    # irfft2 of the constant 0.5 field of atten (sigmoid = 0.5 + 0.5tanh):
    # after ifft over i it is 8*(1+1j) at m=0; irfft over j gives this
    # w-profile on the h=0 row.
    global _CORR
    if _CORR is None:
        AR = np.fft.irfft(np.eye(WF), n=W, axis=0, norm="ortho")
        AI = np.fft.irfft(1j * np.eye(WF), n=W, axis=0, norm="ortho")
        _CORR = 8.0 * (AR.sum(axis=1) + AI.sum(axis=1))
    return _CORR


def build_consts():
    Fw = np.fft.rfft(np.eye(W), axis=0, norm="ortho").T  # [W, WF] complex
    fw_pack = np.zeros((W2, W2), np.float32)
    fw_pack[:W, :WF] = Fw.real
    fw_pack[:W, WF:] = Fw.imag
    # pair-interleaved for DoubleRow: fw8[k, j, n] = fw_pack[2k+j, n] * CS
    fw8 = (fw_pack * CS).reshape(WF, 2, W2)
    DH = np.fft.fft(np.eye(H), axis=0, norm="ortho")  # [kh, h]
    dht_r = np.ascontiguousarray(DH.real.T * CS).astype(E4M3)  # [h, kh]
    dht_i = np.ascontiguousarray(DH.imag.T * CS).astype(E4M3)
    dht_ni = np.ascontiguousarray((-DH.imag).T * CS).astype(E4M3)
    IDH = np.fft.ifft(np.eye(H), axis=0, norm="ortho")  # [m, i]
    ida = np.zeros((H, W2), np.float32)
    ida[:, :WF] = IDH.real.T * (CS * 0.5)
    ida[:, WF:] = IDH.imag.T * (CS * 0.5)
    idb = np.zeros((H, W2), np.float32)
    idb[:, :WF] = -IDH.imag.T * (CS * 0.5)
    idb[:, WF:] = IDH.real.T * (CS * 0.5)
    AR = np.fft.irfft(np.eye(WF), n=W, axis=0, norm="ortho")  # [w, j]
    AI = np.fft.irfft(1j * np.eye(WF), n=W, axis=0, norm="ortho")
    awr = np.zeros((WF, W2), np.float32)
    awr[:, :W] = AR.T * CS
    awi = np.zeros((WF, W2), np.float32)
    awi[:, :W] = AI.T * CS
    return dict(
        fw8=fw8.astype(E4M3),
        dht_r=dht_r, dht_i=dht_i, dht_ni=dht_ni,
        idht_a=ida.astype(E4M3), idht_b=idb.astype(E4M3),
        awr8=awr.astype(E4M3), awi8=awi.astype(E4M3),
    )


def build_program():
    nc = bacc.Bacc("TRN2", target_bir_lowering=False, debug=False,
                   num_devices=N_CORES)

    def inp(name, shape, dt=F32):
        return nc.dram_tensor(name, list(shape), dt, kind="ExternalInput").ap()

    g8 = inp("g8", (C, H, W2), F8)
    x8 = inp("x8", (C, H, W2), F8)
    wgT8 = inp("wgT8", (C, NCH), F8)
    wxT8 = inp("wxT8", (C, NCH), F8)
    bg = inp("bg", (128, 1))
    bx = inp("bx", (128, 1))
    fw8 = inp("fw8", (WF, 2, W2), F8)
    dht_r = inp("dht_r", (H, H), F8)
    dht_i = inp("dht_i", (H, H), F8)
    dht_ni = inp("dht_ni", (H, H), F8)
    idht_a = inp("idht_a", (H, W2), F8)
    idht_b = inp("idht_b", (H, W2), F8)
    awr8 = inp("awr8", (WF, W2), F8)
    awi8 = inp("awi8", (WF, W2), F8)
    # filters packed per channel-pair: [16, 4(c,ri), 256kh, 256kw], x(1/CS)
    fpg = inp("fpg", (NCH // 2, 4, H, WF), F16)
    fpx = inp("fpx", (NCH // 2, 4, H, WF), F16)
    xres = inp("xres", (NCH, H, W2), F16)
    gamma = inp("gamma", (1, NCH))
    beta = inp("beta", (1, NCH))
    out = nc.dram_tensor("out", [NCH, H, W2], F16, kind="ExternalOutput").ap()

    with tile.TileContext(nc) as tc:
        with (
            tc.tile_pool(name="consts", bufs=1) as consts,
            tc.tile_pool(name="dram", bufs=1, space="DRAM") as dram,
        ):
            # ---- constants into SBUF
            c_fw = consts.tile([128, 2, 2, W2], F8, tag="c_fw")
            nc.sync.dma_start(
                c_fw, fw8.rearrange("(kc p) j n -> p kc j n", p=128))

            def ld2(src, ncol=H):
                t = consts.tile([128, 2, ncol], F8, tag=f"c_{src.name}")
                nc.sync.dma_start(t, src.rearrange("(hc p) m -> p hc m", p=128))
                return t

            c_dhtr = ld2(dht_r)
            c_dhti = ld2(dht_i)
            c_dhtni = ld2(dht_ni)
            c_ida = ld2(idht_a, W2)
            c_idb = ld2(idht_b, W2)
            c_awr = ld2(awr8, W2)
            c_awi = ld2(awi8, W2)
            c_wgT = consts.tile([C, NCH], F8, tag="c_wgT")
            nc.sync.dma_start(c_wgT, wgT8)
            c_wxT = consts.tile([C, NCH], F8, tag="c_wxT")
            nc.sync.dma_start(c_wxT, wxT8)
            c_bg = consts.tile([128, 1], F32, tag="c_bg")
            nc.sync.dma_start(c_bg, bg)
            c_bx = consts.tile([128, 1], F32, tag="c_bx")
            nc.sync.dma_start(c_bx, bx)
            c_gamma = consts.tile([128, NCH], F32, tag="c_gamma")
            nc.sync.dma_start(c_gamma, gamma.to_broadcast([128, NCH]))
            c_beta = consts.tile([128, NCH], F32, tag="c_beta")
            nc.sync.dma_start(c_beta, beta.to_broadcast([128, NCH]))

            # ---- DRAM scratch (y spill stored as uint16 fp8-pairs)
            ysp_g = dram.tile([NCH, H, WF], U16, tag="ysp_g")
            ysp_x = dram.tile([NCH, H, WF], U16, tag="ysp_x")
            cc_in = dram.tile([128, 2048], F32, tag="cc_in")
            cc_out = dram.tile([128, 2048], F32, tag="cc_out")

            # ---- Phase A: 1x1 conv (fp8), spill y fp8
            HB = 32
            with (
                tc.tile_pool(name="pa_in", bufs=3) as pa_in,
                tc.tile_pool(name="pa_out", bufs=3) as pa_out,
                tc.tile_pool(name="pa_ps", bufs=4, space="PSUM") as pa_ps,
            ):
                for srct, wTt, biast, yspt in (
                    (g8, c_wgT, c_bg, ysp_g),
                    (x8, c_wxT, c_bx, ysp_x),
                ):
                    for blk in range(H // HB):
                        h0 = blk * HB
                        rh = pa_in.tile([C, HB, W2], F8, tag="rh")
                        nc.sync.dma_start(rh, srct[:, h0:h0 + HB, :])
                        stag = pa_out.tile([128, HB // 4, W2], F8, tag="stag")
                        for i2 in range(HB // 4):
                            ps = pa_ps.tile([128, W2], F32, tag="cps")
                            for j in range(4):
                                nc.tensor.matmul(
                                    ps[32 * j:32 * (j + 1), :], wTt,
                                    rh[:, i2 * 4 + j, :],
                                    start=True, stop=True,
                                    tile_position=(0, 32 * j))
                            if i2 % 2 == 0:
                                nc.scalar.activation(
                                    stag[:, i2, :], ps, AF.Identity,
                                    bias=biast, scale=1.0 / CS)
                            else:
                                nc.vector.tensor_scalar(
                                    out=stag[:, i2, :], in0=ps,
                                    scalar1=1.0 / CS, scalar2=biast,
                                    op0=OP.mult, op1=OP.add)
                        stag16 = stag.bitcast(U16)
                        for j in range(4):
                            nc.sync.dma_start(
                                yspt[:, h0 + j:h0 + HB:4, :],
                                stag16[32 * j:32 * (j + 1), :, :])

            # ---- Phase B: software-pipelined per-channel pipeline
            with (
                tc.tile_pool(name="pb_yt", bufs=2) as pb_yt,
                tc.tile_pool(name="pb_yw", bufs=4) as pb_yw,
                tc.tile_pool(name="pb_zp", bufs=4) as pb_zp,
                tc.tile_pool(name="pb_z", bufs=3) as pb_z,
                tc.tile_pool(name="pb_f", bufs=2) as pb_f,
                tc.tile_pool(name="pb_sm", bufs=3) as pb_sm,
                tc.tile_pool(name="pb_att", bufs=3) as pb_att,
                tc.tile_pool(name="pb_inv", bufs=3) as pb_inv,
                tc.tile_pool(name="pb_x", bufs=2) as pb_x,
                tc.tile_pool(name="pb_acc", bufs=1) as pb_acc,
                tc.tile_pool(name="pb_pw", bufs=2, space="PSUM") as pb_pw,
                tc.tile_pool(name="pb_pf", bufs=2, space="PSUM") as pb_pf,
                tc.tile_pool(name="pb_pt", bufs=4, space="PSUM") as pb_pt,
            ):
                S1 = pb_acc.tile([128, 2, W2], F32, tag="S1")
                S2 = pb_acc.tile([128, 2, W2], F32, tag="S2")
                nc.vector.memset(S1, 0.0)
                nc.vector.memset(S2, 0.0)
                r_all = pb_acc.tile([128, NCH, 2, W2], F16, tag="r_all")

                ytT = {}
                fP = {}
                xc_d = {}

                def frontend(c):
                    cc = c % 2
                    if cc == 0:
                        for t, ysp, fpd in ((0, ysp_g, fpg), (1, ysp_x, fpx)):
                            yt = pb_yt.tile([128, 2, 2, WF], U16, tag=f"yt{t}")
                            for kc in range(2):
                                nc.sync.dma_start_transpose(
                                    yt[:, kc, :, :].rearrange(
                                        "p c h -> p (c h)"),
                                    ysp[c:c + 2, :,
                                        kc * 128:(kc + 1) * 128].rearrange(
                                        "c h w -> (c h) w"))
                            ytT[t] = yt
                            fpt = pb_f.tile([128, 4, 2, WF], F16, tag=f"fp{t}")
                            nc.sync.dma_start(
                                fpt, fpd[c // 2].rearrange(
                                    "cr (khc p) k -> p cr khc k", p=128))
                            fP[t] = fpt
                        xc = pb_x.tile([128, 2, 2, W2], F16, tag="xc")
                        nc.sync.dma_start(
                            xc, xres[c:c + 2].rearrange(
                                "c (mc p) w -> p c mc w", p=128))
                        xc_d[c // 2] = xc
                    z = {}
                    for t in (0, 1):
                        yt8 = ytT[t].bitcast(F8)  # [128, 2kc, 2c, 512]
                        # B1: rfft_w as fp8 DoubleRow over w-pairs
                        yw8 = pb_yw.tile([128, 2, W2], F8, tag="yw8")
                        for hc in range(2):
                            pw = pb_pw.tile([128, W2], F32, tag="pw")
                            first = True
                            for kc in range(2):
                                lhsv = yt8[:, kc, cc, :].rearrange(
                                    "p (h j) -> p j h", j=2)
                                for j in range(2):
                                    nc.tensor.matmul(
                                        pw,
                                        lhsv[:, j,
                                             hc * 128:(hc + 1) * 128],
                                        c_fw[:, kc, j, :],
                                        start=first,
                                        stop=(kc == 1 and j == 1))
                                    first = False
                            nc.scalar.activation(
                                yw8[:, hc, :], pw, AF.Identity,
                                scale=1.0 / CS)
                        # B2: fft_h as fp8 DoubleRow, R and I into one bank
                        ywR = yw8[:, :, 0:WF]
                        ywI = yw8[:, :, WF:W2]
                        zP = pb_zp.tile([128, 2, W2], F16, tag="zP")
                        for khc in range(2):
                            pf = pb_pf.tile([128, W2], F32, tag="pf")
                            ksl = slice(khc * 128, (khc + 1) * 128)
                            nc.tensor.matmul(
                                pf[:, 0:WF], c_dhtr[:, :, ksl], ywR,
                                start=True, stop=False, perf_mode=DR,
                                skip_group_check=True)
                            nc.tensor.matmul(
                                pf[:, 0:WF], c_dhtni[:, :, ksl], ywI,
                                start=False, stop=False, perf_mode=DR,
                                skip_group_check=True)
                            nc.tensor.matmul(
                                pf[:, WF:W2], c_dhti[:, :, ksl], ywR,
                                start=False, stop=False, perf_mode=DR,
                                skip_group_check=True)
                            nc.tensor.matmul(
                                pf[:, WF:W2], c_dhtr[:, :, ksl], ywI,
                                start=False, stop=True, perf_mode=DR,
                                skip_group_check=True)
                            nc.scalar.copy(out=zP[:, khc, :], in_=pf)
                        # B3: filter multiply (fp16 2x TT ops)
                        zRp = zP[:, :, 0:WF]
                        zIp = zP[:, :, WF:W2]
                        fR = fP[t][:, 2 * cc, :, :]
                        fI = fP[t][:, 2 * cc + 1, :, :]
                        t1 = pb_sm.tile([128, 2, WF], F16, tag="t1")
                        t2 = pb_sm.tile([128, 2, WF], F16, tag="t2")
                        t3 = pb_sm.tile([128, 2, WF], F16, tag="t3")
                        t4 = pb_sm.tile([128, 2, WF], F16, tag="t4")
                        nc.vector.tensor_mul(t1, zRp, fR)
                        nc.vector.tensor_mul(t2, zIp, fI)
                        nc.vector.tensor_mul(t3, zRp, fI)
                        nc.vector.tensor_mul(t4, zIp, fR)
                        zR = pb_z.tile([128, 2, WF], F16, tag=f"zR{t}")
                        zI = pb_z.tile([128, 2, WF], F16, tag=f"zI{t}")
                        nc.vector.tensor_sub(zR, t1, t2)
                        nc.vector.tensor_add(zI, t3, t4)
                        z[(t, "R")] = zR
                        z[(t, "I")] = zI
                        if t == 1:
                            nzI = pb_z.tile([128, 2, WF], F16, tag="nzI")
                            nc.vector.tensor_scalar_mul(nzI, zI, -1.0)
                            z[(1, "nI")] = nzI
                    # B4: variance over kw per kh row -> rstd (Pool)
                    gR, gI = z[(0, "R")], z[(0, "I")]
                    v2 = pb_sm.tile([128, 2], F32, tag="v2")
                    for khc in range(2):
                        st = pb_sm.tile([128, 2, 6], F32, tag="bst")
                        nc.vector.bn_stats(out=st[:, 0, :], in_=gR[:, khc, :])
                        nc.vector.bn_stats(out=st[:, 1, :], in_=gI[:, khc, :])
                        mvR = pb_sm.tile([128, 2], F32, tag="mvR")
                        mvI = pb_sm.tile([128, 2], F32, tag="mvI")
                        nc.vector.bn_aggr(out=mvR, in_=st[:, 0, :])
                        nc.vector.bn_aggr(out=mvI, in_=st[:, 1, :])
                        nc.gpsimd.tensor_add(v2[:, khc:khc + 1],
                                             mvR[:, 1:2], mvI[:, 1:2])
                    ti = pb_sm.tile([128, 2], I32, tag="ti")
                    nc.vector.tensor_scalar(
                        out=ti, in0=v2.bitcast(I32), scalar1=1, scalar2=0,
                        op0=OP.arith_shift_right, op1=OP.bypass)
                    nc.vector.tensor_scalar(
                        out=ti, in0=ti, scalar1=-1, scalar2=RSQRT_MAGIC,
                        op0=OP.mult, op1=OP.add)
                    y0 = ti.bitcast(F32)
                    tn = pb_sm.tile([128, 2], F32, tag="tn")
                    nc.gpsimd.tensor_mul(tn, y0, y0)
                    nc.gpsimd.tensor_mul(tn, tn, v2)
                    nc.vector.tensor_scalar(
                        out=tn, in0=tn, scalar1=-0.5, scalar2=1.5,
                        op0=OP.mult, op1=OP.add)
                    rstd = pb_sm.tile([128, 2], F32, tag="rstd")
                    nc.gpsimd.tensor_mul(rstd, y0, tn)
                    nc.vector.tensor_scalar_mul(rstd, rstd,
                                                INV_SQRT_2PI * 0.5)
                    return z, rstd

                def backend1(c, z, rstd):
                    gR, gI = z[(0, "R")], z[(0, "I")]
                    xR, xI, nxI = z[(1, "R")], z[(1, "I")], z[(1, "nI")]
                    # B5: scores (fp16) into one PSUM bank + tanh -> fp8
                    a8 = pb_att.tile([128, 2, W2], F8, tag="a8")
                    for ic in range(2):
                        psc = pb_pt.tile([128, W2], F32, tag="pt")
                        isl = slice(ic * 128, (ic + 1) * 128)
                        for khc in range(2):
                            nc.tensor.matmul(
                                psc[:, 0:WF], gR[:, khc, isl], xR[:, khc, :],
                                start=(khc == 0), stop=False,
                                skip_group_check=True)
                            nc.tensor.matmul(
                                psc[:, 0:WF], gI[:, khc, isl], nxI[:, khc, :],
                                start=False, stop=False,
                                skip_group_check=True)
                            nc.tensor.matmul(
                                psc[:, WF:W2], gR[:, khc, isl], xI[:, khc, :],
                                start=False, stop=False,
                                skip_group_check=True)
                            nc.tensor.matmul(
                                psc[:, WF:W2], gI[:, khc, isl], xR[:, khc, :],
                                start=False, stop=(khc == 1),
                                skip_group_check=True)
                        nc.scalar.activation(a8[:, ic, :], psc, AF.Tanh,
                                             scale=rstd[:, ic:ic + 1])
                    return a8

                def backend2(c, a8):
                    cc = c % 2
                    xc = xc_d[c // 2]
                    # B7: ifft over i (fp8 DoubleRow), inv = pv/16
                    inv8 = pb_inv.tile([128, 2, W2], F8, tag="inv8")
                    for jc in range(2):
                        pv = pb_pt.tile([128, W2], F32, tag="pt")
                        jsl = slice(jc * 128, (jc + 1) * 128)
                        jsl2 = slice(WF + jc * 128, WF + (jc + 1) * 128)
                        nc.tensor.matmul(pv, a8[:, :, jsl], c_ida,
                                         start=True, stop=False, perf_mode=DR)
                        nc.tensor.matmul(pv, a8[:, :, jsl2], c_idb,
                                         start=False, stop=True, perf_mode=DR)
                        nc.scalar.activation(inv8[:, jc, :], pv, AF.Identity,
                                             scale=1.0 / CS)
                    # B8: irfft over j (fp8 DoubleRow) + residual + stats
                    for mc in range(2):
                        pr = pb_pt.tile([128, W2], F32, tag="pt")
                        msl = slice(mc * 128, (mc + 1) * 128)
                        msl2 = slice(WF + mc * 128, WF + (mc + 1) * 128)
                        nc.tensor.matmul(pr, inv8[:, :, msl], c_awr,
                                         start=True, stop=False, perf_mode=DR)
                        nc.tensor.matmul(pr, inv8[:, :, msl2], c_awi,
                                         start=False, stop=True, perf_mode=DR)
                        rc = r_all[:, c, mc, :]
                        nc.vector.scalar_tensor_tensor(
                            out=rc, in0=pr, scalar=1.0 / CS,
                            in1=xc[:, cc, mc, :], op0=OP.mult, op1=OP.add)
                        nc.gpsimd.tensor_add(S1[:, mc, :], S1[:, mc, :], rc)
                        sq = pb_sm.tile([128, W2], F16, tag="sq")
                        nc.scalar.activation(sq, rc, AF.Square,
                                             scale=1.0 / 16.0)
                        nc.gpsimd.tensor_add(S2[:, mc, :], S2[:, mc, :], sq)

                state = {}
                att_st = {}
                for c in range(NCH + 2):
                    if c < NCH:
                        state[c] = frontend(c)
                    if 1 <= c <= NCH:
                        att_st[c - 1] = backend1(c - 1, *state.pop(c - 1))
                    if c >= 2:
                        backend2(c - 2, att_st.pop(c - 2))

                # ---- Phase C: LN stats AllReduce + normalize
                nc.sync.dma_start(cc_in[:, 0:1024],
                                  S1.rearrange("p a b -> p (a b)"))
                nc.sync.dma_start(cc_in[:, 1024:2048],
                                  S2.rearrange("p a b -> p (a b)"))
                nc.gpsimd.collective_compute(
                    "AllReduce", OP.add,
                    replica_groups=[[0, 1, 2, 3], [4, 5, 6, 7]],
                    ins=[cc_in.opt()], outs=[cc_out.opt()])
                mu32 = pb_acc.tile([128, 1024], F32, tag="mu32")
                nc.sync.dma_start(mu32, cc_out[:, 0:1024])
                nc.vector.tensor_scalar_mul(mu32, mu32, 1.0 / C)
                e2 = pb_acc.tile([128, 1024], F32, tag="e2")
                nc.sync.dma_start(e2, cc_out[:, 1024:2048])
                nc.vector.tensor_scalar_mul(e2, e2, 256.0 / C)
                var = pb_acc.tile([128, 1024], F32, tag="var")
                nc.vector.scalar_tensor_tensor(
                    out=var, in0=mu32, scalar=-1.0, in1=mu32,
                    op0=OP.mult, op1=OP.mult)
                nc.vector.scalar_tensor_tensor(
                    out=var, in0=var, scalar=1e-6, in1=e2,
                    op0=OP.add, op1=OP.add)
                tiL = pb_acc.tile([128, 1024], I32, tag="tiL")
                nc.vector.tensor_scalar(
                    out=tiL, in0=var.bitcast(I32), scalar1=1, scalar2=0,
                    op0=OP.arith_shift_right, op1=OP.bypass)
                nc.vector.tensor_scalar(
                    out=tiL, in0=tiL, scalar1=-1, scalar2=RSQRT_MAGIC,
                    op0=OP.mult, op1=OP.add)
                y0L = tiL.bitcast(F32)
                tnL = pb_acc.tile([128, 1024], F32, tag="tnL")
                rsL = pb_acc.tile([128, 1024], F32, tag="rsL")
                for it in range(2):
                    nc.vector.scalar_tensor_tensor(
                        out=tnL, in0=y0L, scalar=1.0, in1=y0L,
                        op0=OP.bypass, op1=OP.mult)
                    nc.vector.scalar_tensor_tensor(
                        out=tnL, in0=tnL, scalar=1.0, in1=var,
                        op0=OP.bypass, op1=OP.mult)
                    nc.vector.tensor_scalar(
                        out=tnL, in0=tnL, scalar1=-0.5, scalar2=1.5,
                        op0=OP.mult, op1=OP.add)
                    nc.vector.scalar_tensor_tensor(
                        out=rsL, in0=y0L, scalar=1.0, in1=tnL,
                        op0=OP.bypass, op1=OP.mult)
                    y0L = rsL
                mu16 = pb_acc.tile([128, 2, W2], F16, tag="mu16")
                nc.vector.tensor_copy(
                    out=mu16.rearrange("p a b -> p (a b)"), in_=mu32)
                rs16 = pb_acc.tile([128, 2, W2], F16, tag="rs16")
                nc.vector.tensor_copy(
                    out=rs16.rearrange("p a b -> p (a b)"), in_=rsL)
                with tc.tile_pool(name="pc_o", bufs=3) as pc_o:
                    for c in range(NCH):
                        ob = pc_o.tile([128, 2, W2], F16, tag="ob")
                        for mc in range(2):
                            tt = pc_o.tile([128, W2], F16, tag="tt")
                            nc.gpsimd.tensor_sub(tt, r_all[:, c, mc, :],
                                                 mu16[:, mc, :])
                            nc.vector.tensor_mul(tt, tt, rs16[:, mc, :])
                            nc.vector.tensor_scalar(
                                out=ob[:, mc, :], in0=tt,
                                scalar1=c_gamma[:, c:c + 1],
                                scalar2=c_beta[:, c:c + 1],
                                op0=OP.mult, op1=OP.add)
                        nc.sync.dma_start(
                            out[c].rearrange("(mc p) w -> p mc w", p=128), ob)
    nc.compile()
    return nc


_PROGRAM = None


def kernel(_trace=False, **inputs):
    global _PROGRAM
    np_in = {k: np.ascontiguousarray(np.asarray(v)) for k, v in inputs.items()}
    g, x = np_in["g"], np_in["x"]
    consts = build_consts()

    def pack_gx(a):
        p = np.zeros((C, H, W2), E4M3)
        p[:, :, :W] = a.astype(E4M3)
        return p

    def pack_w(wc, sl):
        return np.ascontiguousarray((wc[sl].T * CS).astype(E4M3))

    def pack_filt(f):
        # f [32, H, WF, 2] -> [16, 4(c,ri), H, WF] fp16, pre-divided by CS
        # to undo the x16 fp8 scaling of the dht constants.
        m = np.moveaxis(f, 3, 1) * (1.0 / CS)  # [32, 2, H, WF]
        return np.ascontiguousarray(
            m.reshape(16, 4, H, WF).astype(np.float16))

    in_maps = []
    for k in range(N_CORES):
        b, grp = k // 4, k % 4
        sl = slice(grp * NCH, (grp + 1) * NCH)
        xr = np.zeros((NCH, H, W2), np.float16)
        xr[:, :, :W] = x[b][sl].astype(np.float16)
        xr[:, 0, :W] += _corr_w().astype(np.float16)
        m = dict(
            g8=pack_gx(g[b]),
            x8=pack_gx(x[b]),
            wgT8=pack_w(np_in["wg_conv"], sl),
            wxT8=pack_w(np_in["wx_conv"], sl),
            bg=np.ascontiguousarray(
                np.tile(np_in["bg_conv"][sl], 4)[:, None]).astype(np.float32),
            bx=np.ascontiguousarray(
                np.tile(np_in["bx_conv"][sl], 4)[:, None]).astype(np.float32),
            fpg=pack_filt(np_in["filt_g"][sl]),
            fpx=pack_filt(np_in["filt_x"][sl]),
            xres=xr,
            gamma=np.ascontiguousarray(
                np_in["ln_gamma"][sl][None, :]).astype(np.float32),
            beta=np.ascontiguousarray(
                np_in["ln_beta"][sl][None, :]).astype(np.float32),
            **consts,
        )
        in_maps.append(m)
    if _PROGRAM is None:
        _PROGRAM = build_program()
    res = run_bass_kernel_spmd(_PROGRAM, in_maps, core_ids=list(range(N_CORES)),
                               trace=_trace)
    outf = np.zeros((B, C, H, W), np.float32)
    for k in range(N_CORES):
        b, grp = k // 4, k % 4
        outf[b, grp * NCH:(grp + 1) * NCH] = \
            res.results[k]["out"][:, :, :W].astype(np.float32)
    if _trace:
        kernel.last_results = res
    return outf


if __name__ == "__main__":
    ins = {
        "g": np.random.randn(B, C, H, W).astype(np.float32),
        "x": np.random.randn(B, C, H, W).astype(np.float32),
        "wg_conv": (np.random.randn(C, C) * 0.05).astype(np.float32),
        "bg_conv": np.zeros(C, np.float32),
        "wx_conv": (np.random.randn(C, C) * 0.05).astype(np.float32),
        "bx_conv": np.zeros(C, np.float32),
        "filt_g": (np.random.randn(C, H, WF, 2) * 0.02).astype(np.float32),
        "filt_x": (np.random.randn(C, H, WF, 2) * 0.02).astype(np.float32),
        "ln_gamma": np.ones(C, np.float32),
        "ln_beta": np.zeros(C, np.float32),
    }
    o = kernel(**ins)
    print("kernel ran, out shape", o.shape)


# revision 24
# speedup vs baseline: 1.0163x; 1.0163x over previous
"""Trainium2 Bass kernel for nn_AttentionFilter (B=2,C=128,H=256,W=510).

Sharding: 8 cores = 2 batches x 4 channel-groups of 32. Per core:
  Phase A: 1x1 conv as fp8 matmul (x16-scaled fp8 weights, 1/16+bias fused
    into the PSUM->SBUF copy), spill y as fp8 to DRAM padded to W=512.
  Phase B (software-pipelined, backend of channel c-1 interleaved with
    frontend of channel c): xbar DMA-transpose of fp8 y viewed as uint16
    pairs -> pair-interleaved [w/2, 2, h] stationaries; rfft_w and fft_h as
    fp8 DoubleRow matmuls (x16-scaled fp8 DFT constants, rescale fused into
    PSUM copies); complex filter multiply as fp16 2x-mode tensor_tensor;
    variance via bn_stats + integer-magic rsqrt on Pool; freq attention
    matmul fp16 into a single PSUM bank; tanh on ACT (sigmoid = 0.5 +
    0.5tanh, the 0.5-DC correction folded into xres on host) with fused
    rstd/2 row scale writing fp8; ifft_i and irfft_j as fp8 DoubleRow
    matmuls; residual add fused with 1/16 rescale; LN stats: S1/S2 fp32
    accumulated on Pool, squares via ACT (x1/256 to bound range).
  Phase C: fp32 AllReduce of LN stats within each batch group, rsqrt via
    integer magic + 2 Newton steps, per-channel normalize, fp16 output.
"""
import sys

sys.path.insert(0, "/opt/trn_rl_repo")

import numpy as np
import ml_dtypes

import concourse.bass as bass
import concourse.mybir as mybir
import concourse.tile as tile
from concourse import bacc
from concourse.bass_utils import run_bass_kernel_spmd

B, C, H, W = 2, 128, 256, 510
WF = 256
W2 = 512
NCH = 32  # channels per core
N_CORES = 8
F32 = mybir.dt.float32
F16 = mybir.dt.float16
F8 = mybir.dt.float8e4
U16 = mybir.dt.uint16
I32 = mybir.dt.int32
AF = mybir.ActivationFunctionType
OP = mybir.AluOpType
DR = mybir.MatmulPerfMode.DoubleRow

E4M3 = ml_dtypes.float8_e4m3
CS = 16.0  # fp8 constant scale
RSQRT_MAGIC = 0x5F3759DF
INV_SQRT_2PI = float(1.0 / np.sqrt(2.0 * np.pi))

_CORR = None


def _corr_w():
    # irfft2 of the constant 0.5 field of atten (sigmoid = 0.5 + 0.5tanh):
    # after ifft over i it is 8*(1+1j) at m=0; irfft over j gives this
    # w-profile on the h=0 row.
    global _CORR
    if _CORR is None:
        AR = np.fft.irfft(np.eye(WF), n=W, axis=0, norm="ortho")
        AI = np.fft.irfft(1j * np.eye(WF), n=W, axis=0, norm="ortho")
        _CORR = 8.0 * (AR.sum(axis=1) + AI.sum(axis=1))
    return _CORR


def build_consts():
    Fw = np.fft.rfft(np.eye(W), axis=0, norm="ortho").T  # [W, WF] complex
    fw_pack = np.zeros((W2, W2), np.float32)
    fw_pack[:W, :WF] = Fw.real
    fw_pack[:W, WF:] = Fw.imag
    # pair-interleaved for DoubleRow: fw8[k, j, n] = fw_pack[2k+j, n] * CS
    fw8 = (fw_pack * CS).reshape(WF, 2, W2)
    DH = np.fft.fft(np.eye(H), axis=0, norm="ortho")  # [kh, h]
    dht_r = np.ascontiguousarray(DH.real.T * CS).astype(E4M3)  # [h, kh]
    dht_i = np.ascontiguousarray(DH.imag.T * CS).astype(E4M3)
    dht_ni = np.ascontiguousarray((-DH.imag).T * CS).astype(E4M3)
    IDH = np.fft.ifft(np.eye(H), axis=0, norm="ortho")  # [m, i]
    ida = np.zeros((H, W2), np.float32)
    ida[:, :WF] = IDH.real.T * (CS * 0.5)
    ida[:, WF:] = IDH.imag.T * (CS * 0.5)
    idb = np.zeros((H, W2), np.float32)
    idb[:, :WF] = -IDH.imag.T * (CS * 0.5)
    idb[:, WF:] = IDH.real.T * (CS * 0.5)
    AR = np.fft.irfft(np.eye(WF), n=W, axis=0, norm="ortho")  # [w, j]
    AI = np.fft.irfft(1j * np.eye(WF), n=W, axis=0, norm="ortho")
    awr = np.zeros((WF, W2), np.float32)
    awr[:, :W] = AR.T * CS
    awi = np.zeros((WF, W2), np.float32)
    awi[:, :W] = AI.T * CS
    return dict(
        fw8=fw8.astype(E4M3),
        dht_r=dht_r, dht_i=dht_i, dht_ni=dht_ni,
        idht_a=ida.astype(E4M3), idht_b=idb.astype(E4M3),
        awr8=awr.astype(E4M3), awi8=awi.astype(E4M3),
    )


def build_program():
    nc = bacc.Bacc("TRN2", target_bir_lowering=False, debug=False,
                   num_devices=N_CORES)

    def inp(name, shape, dt=F32):
        return nc.dram_tensor(name, list(shape), dt, kind="ExternalInput").ap()

    g8 = inp("g8", (C, H, W2), F8)
    x8 = inp("x8", (C, H, W2), F8)
    wgT8 = inp("wgT8", (C, NCH), F8)
    wxT8 = inp("wxT8", (C, NCH), F8)
    bg = inp("bg", (128, 1))
    bx = inp("bx", (128, 1))
    fw8 = inp("fw8", (WF, 2, W2), F8)
    dht_r = inp("dht_r", (H, H), F8)
    dht_i = inp("dht_i", (H, H), F8)
    dht_ni = inp("dht_ni", (H, H), F8)
    idht_a = inp("idht_a", (H, W2), F8)
    idht_b = inp("idht_b", (H, W2), F8)
    awr8 = inp("awr8", (WF, W2), F8)
    awi8 = inp("awi8", (WF, W2), F8)
    # filters packed per channel-pair: [16, 4(c,ri), 256kh, 256kw], x(1/CS)
    fpg = inp("fpg", (NCH // 2, 4, H, WF), F16)
    fpx = inp("fpx", (NCH // 2, 4, H, WF), F16)
    xres = inp("xres", (NCH, H, W2), F16)
    gamma = inp("gamma", (1, NCH))
    beta = inp("beta", (1, NCH))
    out = nc.dram_tensor("out", [NCH, H, W2], F16, kind="ExternalOutput").ap()

    with tile.TileContext(nc) as tc:
        with (
            tc.tile_pool(name="consts", bufs=1) as consts,
            tc.tile_pool(name="dram", bufs=1, space="DRAM") as dram,
        ):
            # ---- constants into SBUF
            c_fw = consts.tile([128, 2, 2, W2], F8, tag="c_fw")
            nc.sync.dma_start(
                c_fw, fw8.rearrange("(kc p) j n -> p kc j n", p=128))

            def ld2(src, ncol=H):
                t = consts.tile([128, 2, ncol], F8, tag=f"c_{src.name}")
                nc.sync.dma_start(t, src.rearrange("(hc p) m -> p hc m", p=128))
                return t

            c_dhtr = ld2(dht_r)
            c_dhti = ld2(dht_i)
            c_dhtni = ld2(dht_ni)
            c_ida = ld2(idht_a, W2)
            c_idb = ld2(idht_b, W2)
            c_awr = ld2(awr8, W2)
            c_awi = ld2(awi8, W2)
            c_wgT = consts.tile([C, NCH], F8, tag="c_wgT")
            nc.sync.dma_start(c_wgT, wgT8)
            c_wxT = consts.tile([C, NCH], F8, tag="c_wxT")
            nc.sync.dma_start(c_wxT, wxT8)
            c_bg = consts.tile([128, 1], F32, tag="c_bg")
            nc.sync.dma_start(c_bg, bg)
            c_bx = consts.tile([128, 1], F32, tag="c_bx")
            nc.sync.dma_start(c_bx, bx)
            c_gamma = consts.tile([128, NCH], F32, tag="c_gamma")
            nc.sync.dma_start(c_gamma, gamma.to_broadcast([128, NCH]))
            c_beta = consts.tile([128, NCH], F32, tag="c_beta")
            nc.sync.dma_start(c_beta, beta.to_broadcast([128, NCH]))

            # ---- DRAM scratch (y spill stored as uint16 fp8-pairs)
            ysp_g = dram.tile([NCH, H, WF], U16, tag="ysp_g")
            ysp_x = dram.tile([NCH, H, WF], U16, tag="ysp_x")
            cc_in = dram.tile([128, 2048], F32, tag="cc_in")
            cc_out = dram.tile([128, 2048], F32, tag="cc_out")

            # ---- Phase A: 1x1 conv (fp8), spill y fp8
            HB = 32
            with (
                tc.tile_pool(name="pa_in", bufs=3) as pa_in,
                tc.tile_pool(name="pa_out", bufs=3) as pa_out,
                tc.tile_pool(name="pa_ps", bufs=4, space="PSUM") as pa_ps,
            ):
                for srct, wTt, biast, yspt in (
                    (g8, c_wgT, c_bg, ysp_g),
                    (x8, c_wxT, c_bx, ysp_x),
                ):
                    for blk in range(H // HB):
                        h0 = blk * HB
                        rh = pa_in.tile([C, HB, W2], F8, tag="rh")
                        nc.sync.dma_start(rh, srct[:, h0:h0 + HB, :])
                        stag = pa_out.tile([128, HB // 4, W2], F8, tag="stag")
                        for i2 in range(HB // 4):
                            ps = pa_ps.tile([128, W2], F32, tag="cps")
                            for j in range(4):
                                nc.tensor.matmul(
                                    ps[32 * j:32 * (j + 1), :], wTt,
                                    rh[:, i2 * 4 + j, :],
                                    start=True, stop=True,
                                    tile_position=(0, 32 * j))
                            if i2 % 2 == 0:
                                nc.scalar.activation(
                                    stag[:, i2, :], ps, AF.Identity,
                                    bias=biast, scale=1.0 / CS)
                            else:
                                nc.vector.tensor_scalar(
                                    out=stag[:, i2, :], in0=ps,
                                    scalar1=1.0 / CS, scalar2=biast,
                                    op0=OP.mult, op1=OP.add)
                        stag16 = stag.bitcast(U16)
                        for j in range(4):
                            nc.sync.dma_start(
                                yspt[:, h0 + j:h0 + HB:4, :],
                                stag16[32 * j:32 * (j + 1), :, :])

            # ---- Phase B: software-pipelined per-channel pipeline
            with (
                tc.tile_pool(name="pb_yt", bufs=2) as pb_yt,
                tc.tile_pool(name="pb_yw", bufs=4) as pb_yw,
                tc.tile_pool(name="pb_zp", bufs=4) as pb_zp,
                tc.tile_pool(name="pb_z", bufs=3) as pb_z,
                tc.tile_pool(name="pb_f", bufs=2) as pb_f,
                tc.tile_pool(name="pb_sm", bufs=3) as pb_sm,
                tc.tile_pool(name="pb_att", bufs=3) as pb_att,
                tc.tile_pool(name="pb_inv", bufs=3) as pb_inv,
                tc.tile_pool(name="pb_x", bufs=2) as pb_x,
                tc.tile_pool(name="pb_acc", bufs=1) as pb_acc,
                tc.tile_pool(name="pb_pw", bufs=2, space="PSUM") as pb_pw,
                tc.tile_pool(name="pb_pf", bufs=2, space="PSUM") as pb_pf,
                tc.tile_pool(name="pb_pt", bufs=4, space="PSUM") as pb_pt,
            ):
                S1 = pb_acc.tile([128, 2, W2], F32, tag="S1")
                S2 = pb_acc.tile([128, 2, W2], F32, tag="S2")
                nc.vector.memset(S1, 0.0)
                nc.vector.memset(S2, 0.0)
                r_all = pb_acc.tile([128, NCH, 2, W2], F16, tag="r_all")

                ytT = {}
                fP = {}
                xc_d = {}

                def frontend(c):
                    cc = c % 2
                    if cc == 0:
                        for t, ysp, fpd in ((0, ysp_g, fpg), (1, ysp_x, fpx)):
                            yt = pb_yt.tile([128, 2, 2, WF], U16, tag=f"yt{t}")
                            for kc in range(2):
                                nc.sync.dma_start_transpose(
                                    yt[:, kc, :, :].rearrange(
                                        "p c h -> p (c h)"),
                                    ysp[c:c + 2, :,
                                        kc * 128:(kc + 1) * 128].rearrange(
                                        "c h w -> (c h) w"))
                            ytT[t] = yt
                            fpt = pb_f.tile([128, 4, 2, WF], F16, tag=f"fp{t}")
                            nc.sync.dma_start(
                                fpt, fpd[c // 2].rearrange(
                                    "cr (khc p) k -> p cr khc k", p=128))
                            fP[t] = fpt
                        xc = pb_x.tile([128, 2, 2, W2], F16, tag="xc")
                        nc.sync.dma_start(
                            xc, xres[c:c + 2].rearrange(
                                "c (mc p) w -> p c mc w", p=128))
                        xc_d[c // 2] = xc
                    z = {}
                    for t in (0, 1):
                        yt8 = ytT[t].bitcast(F8)  # [128, 2kc, 2c, 512]
                        # B1: rfft_w as fp8 DoubleRow over w-pairs
                        yw8 = pb_yw.tile([128, 2, W2], F8, tag="yw8")
                        for hc in range(2):
                            pw = pb_pw.tile([128, W2], F32, tag="pw")
                            first = True
                            for kc in range(2):
                                lhsv = yt8[:, kc, cc, :].rearrange(
                                    "p (h j) -> p j h", j=2)
                                for j in range(2):
                                    nc.tensor.matmul(
                                        pw,
                                        lhsv[:, j,
                                             hc * 128:(hc + 1) * 128],
                                        c_fw[:, kc, j, :],
                                        start=first,
                                        stop=(kc == 1 and j == 1))
                                    first = False
                            nc.scalar.activation(
                                yw8[:, hc, :], pw, AF.Identity,
                                scale=1.0 / CS)
                        # B2: fft_h as fp8 DoubleRow, R and I into one bank
                        ywR = yw8[:, :, 0:WF]
                        ywI = yw8[:, :, WF:W2]
                        zP = pb_zp.tile([128, 2, W2], F16, tag="zP")
                        for khc in range(2):
                            pf = pb_pf.tile([128, W2], F32, tag="pf")
                            ksl = slice(khc * 128, (khc + 1) * 128)
                            nc.tensor.matmul(
                                pf[:, 0:WF], c_dhtr[:, :, ksl], ywR,
                                start=True, stop=False, perf_mode=DR,
                                skip_group_check=True)
                            nc.tensor.matmul(
                                pf[:, 0:WF], c_dhtni[:, :, ksl], ywI,
                                start=False, stop=False, perf_mode=DR,
                                skip_group_check=True)
                            nc.tensor.matmul(
                                pf[:, WF:W2], c_dhti[:, :, ksl], ywR,
                                start=False, stop=False, perf_mode=DR,
                                skip_group_check=True)
                            nc.tensor.matmul(
                                pf[:, WF:W2], c_dhtr[:, :, ksl], ywI,
                                start=False, stop=True, perf_mode=DR,
                                skip_group_check=True)
                            nc.scalar.copy(out=zP[:, khc, :], in_=pf)
                        # B3: filter multiply (fp16 2x TT ops)
                        zRp = zP[:, :, 0:WF]
                        zIp = zP[:, :, WF:W2]
                        fR = fP[t][:, 2 * cc, :, :]
                        fI = fP[t][:, 2 * cc + 1, :, :]
                        t1 = pb_sm.tile([128, 2, WF], F16, tag="t1")
                        t2 = pb_sm.tile([128, 2, WF], F16, tag="t2")
                        t3 = pb_sm.tile([128, 2, WF], F16, tag="t3")
                        t4 = pb_sm.tile([128, 2, WF], F16, tag="t4")
                        nc.vector.tensor_mul(t1, zRp, fR)
                        nc.vector.tensor_mul(t2, zIp, fI)
                        nc.vector.tensor_mul(t3, zRp, fI)
                        nc.vector.tensor_mul(t4, zIp, fR)
                        zR = pb_z.tile([128, 2, WF], F16, tag=f"zR{t}")
                        zI = pb_z.tile([128, 2, WF], F16, tag=f"zI{t}")
                        nc.vector.tensor_sub(zR, t1, t2)
                        nc.vector.tensor_add(zI, t3, t4)
                        z[(t, "R")] = zR
                        z[(t, "I")] = zI
                        if t == 1:
                            nzI = pb_z.tile([128, 2, WF], F16, tag="nzI")
                            nc.vector.tensor_scalar_mul(nzI, zI, -1.0)
                            z[(1, "nI")] = nzI
                    # B4: variance over kw per kh row -> rstd (Pool)
                    gR, gI = z[(0, "R")], z[(0, "I")]
                    v2 = pb_sm.tile([128, 2], F32, tag="v2")
                    for khc in range(2):
                        st = pb_sm.tile([128, 2, 6], F32, tag="bst")
                        nc.vector.bn_stats(out=st[:, 0, :], in_=gR[:, khc, :])
                        nc.vector.bn_stats(out=st[:, 1, :], in_=gI[:, khc, :])
                        mvR = pb_sm.tile([128, 2], F32, tag="mvR")
                        mvI = pb_sm.tile([128, 2], F32, tag="mvI")
                        nc.vector.bn_aggr(out=mvR, in_=st[:, 0, :])
                        nc.vector.bn_aggr(out=mvI, in_=st[:, 1, :])
                        nc.gpsimd.tensor_add(v2[:, khc:khc + 1],
                                             mvR[:, 1:2], mvI[:, 1:2])
                    ti = pb_sm.tile([128, 2], I32, tag="ti")
                    nc.vector.tensor_scalar(
                        out=ti, in0=v2.bitcast(I32), scalar1=1, scalar2=0,
                        op0=OP.arith_shift_right, op1=OP.bypass)
                    nc.vector.tensor_scalar(
                        out=ti, in0=ti, scalar1=-1, scalar2=RSQRT_MAGIC,
                        op0=OP.mult, op1=OP.add)
                    y0 = ti.bitcast(F32)
                    tn = pb_sm.tile([128, 2], F32, tag="tn")
                    nc.gpsimd.tensor_mul(tn, y0, y0)
                    nc.gpsimd.tensor_mul(tn, tn, v2)
                    # fold the 0.5/sqrt(2pi) tanh prescale into the Newton
                    # constants: rstd = y0*(1.5 - 0.5*v*y0^2)*K = y0*tn'
                    K = INV_SQRT_2PI * 0.5
                    nc.vector.tensor_scalar(
                        out=tn, in0=tn, scalar1=-0.5 * K, scalar2=1.5 * K,
                        op0=OP.mult, op1=OP.add)
                    rstd = pb_sm.tile([128, 2], F32, tag="rstd")
                    nc.gpsimd.tensor_mul(rstd, y0, tn)
                    return z, rstd

                def backend1(c, z, rstd):
                    gR, gI = z[(0, "R")], z[(0, "I")]
                    xR, xI, nxI = z[(1, "R")], z[(1, "I")], z[(1, "nI")]
                    # B5: scores (fp16) into one PSUM bank + tanh -> fp8
                    a8 = pb_att.tile([128, 2, W2], F8, tag="a8")
                    for ic in range(2):
                        psc = pb_pt.tile([128, W2], F32, tag="pt")
                        isl = slice(ic * 128, (ic + 1) * 128)
                        for khc in range(2):
                            nc.tensor.matmul(
                                psc[:, 0:WF], gR[:, khc, isl], xR[:, khc, :],
                                start=(khc == 0), stop=False,
                                skip_group_check=True)
                            nc.tensor.matmul(
                                psc[:, 0:WF], gI[:, khc, isl], nxI[:, khc, :],
                                start=False, stop=False,
                                skip_group_check=True)
                            nc.tensor.matmul(
                                psc[:, WF:W2], gR[:, khc, isl], xI[:, khc, :],
                                start=False, stop=False,
                                skip_group_check=True)
                            nc.tensor.matmul(
                                psc[:, WF:W2], gI[:, khc, isl], xR[:, khc, :],
                                start=False, stop=(khc == 1),
                                skip_group_check=True)
                        nc.scalar.activation(a8[:, ic, :], psc, AF.Tanh,
                                             scale=rstd[:, ic:ic + 1])
                    return a8

                def backend2(c, a8):
                    cc = c % 2
                    xc = xc_d[c // 2]
                    # B7: ifft over i (fp8 DoubleRow), inv = pv/16
                    inv8 = pb_inv.tile([128, 2, W2], F8, tag="inv8")
                    for jc in range(2):
                        pv = pb_pt.tile([128, W2], F32, tag="pt")
                        jsl = slice(jc * 128, (jc + 1) * 128)
                        jsl2 = slice(WF + jc * 128, WF + (jc + 1) * 128)
                        nc.tensor.matmul(pv, a8[:, :, jsl], c_ida,
                                         start=True, stop=False, perf_mode=DR)
                        nc.tensor.matmul(pv, a8[:, :, jsl2], c_idb,
                                         start=False, stop=True, perf_mode=DR)
                        nc.scalar.activation(inv8[:, jc, :], pv, AF.Identity,
                                             scale=1.0 / CS)
                    # B8: irfft over j (fp8 DoubleRow) + residual + stats
                    for mc in range(2):
                        pr = pb_pt.tile([128, W2], F32, tag="pt")
                        msl = slice(mc * 128, (mc + 1) * 128)
                        msl2 = slice(WF + mc * 128, WF + (mc + 1) * 128)
                        nc.tensor.matmul(pr, inv8[:, :, msl], c_awr,
                                         start=True, stop=False, perf_mode=DR)
                        nc.tensor.matmul(pr, inv8[:, :, msl2], c_awi,
                                         start=False, stop=True, perf_mode=DR)
                        rc = r_all[:, c, mc, :]
                        nc.vector.scalar_tensor_tensor(
                            out=rc, in0=pr, scalar=1.0 / CS,
                            in1=xc[:, cc, mc, :], op0=OP.mult, op1=OP.add)
                        nc.gpsimd.tensor_add(S1[:, mc, :], S1[:, mc, :], rc)
                        sq = pb_sm.tile([128, W2], F16, tag="sq")
                        nc.scalar.activation(sq, rc, AF.Square,
                                             scale=1.0 / 16.0)
                        nc.gpsimd.tensor_add(S2[:, mc, :], S2[:, mc, :], sq)

                state = {}
                att_st = {}
                for c in range(NCH + 2):
                    if c < NCH:
                        state[c] = frontend(c)
                    if 1 <= c <= NCH:
                        att_st[c - 1] = backend1(c - 1, *state.pop(c - 1))
                    if c >= 2:
                        backend2(c - 2, att_st.pop(c - 2))

                # ---- Phase C: LN stats AllReduce + normalize
                nc.sync.dma_start(cc_in[:, 0:1024],
                                  S1.rearrange("p a b -> p (a b)"))
                nc.sync.dma_start(cc_in[:, 1024:2048],
                                  S2.rearrange("p a b -> p (a b)"))
                nc.gpsimd.collective_compute(
                    "AllReduce", OP.add,
                    replica_groups=[[0, 1, 2, 3], [4, 5, 6, 7]],
                    ins=[cc_in.opt()], outs=[cc_out.opt()])
                mu32 = pb_acc.tile([128, 1024], F32, tag="mu32")
                nc.sync.dma_start(mu32, cc_out[:, 0:1024])
                nc.vector.tensor_scalar_mul(mu32, mu32, 1.0 / C)
                e2 = pb_acc.tile([128, 1024], F32, tag="e2")
                nc.sync.dma_start(e2, cc_out[:, 1024:2048])
                nc.vector.tensor_scalar_mul(e2, e2, 256.0 / C)
                var = pb_acc.tile([128, 1024], F32, tag="var")
                nc.vector.scalar_tensor_tensor(
                    out=var, in0=mu32, scalar=-1.0, in1=mu32,
                    op0=OP.mult, op1=OP.mult)
                nc.vector.scalar_tensor_tensor(
                    out=var, in0=var, scalar=1e-6, in1=e2,
                    op0=OP.add, op1=OP.add)
                tiL = pb_acc.tile([128, 1024], I32, tag="tiL")
                nc.vector.tensor_scalar(
                    out=tiL, in0=var.bitcast(I32), scalar1=1, scalar2=0,
                    op0=OP.arith_shift_right, op1=OP.bypass)
                nc.vector.tensor_scalar(
                    out=tiL, in0=tiL, scalar1=-1, scalar2=RSQRT_MAGIC,
                    op0=OP.mult, op1=OP.add)
                y0L = tiL.bitcast(F32)
                tnL = pb_acc.tile([128, 1024], F32, tag="tnL")
                rsL = pb_acc.tile([128, 1024], F32, tag="rsL")
                for it in range(2):
                    nc.vector.scalar_tensor_tensor(
                        out=tnL, in0=y0L, scalar=1.0, in1=y0L,
                        op0=OP.bypass, op1=OP.mult)
                    nc.vector.scalar_tensor_tensor(
                        out=tnL, in0=tnL, scalar=1.0, in1=var,
                        op0=OP.bypass, op1=OP.mult)
                    nc.vector.tensor_scalar(
                        out=tnL, in0=tnL, scalar1=-0.5, scalar2=1.5,
                        op0=OP.mult, op1=OP.add)
                    nc.vector.scalar_tensor_tensor(
                        out=rsL, in0=y0L, scalar=1.0, in1=tnL,
                        op0=OP.bypass, op1=OP.mult)
                    y0L = rsL
                mu16 = pb_acc.tile([128, 2, W2], F16, tag="mu16")
                nc.vector.tensor_copy(
                    out=mu16.rearrange("p a b -> p (a b)"), in_=mu32)
                rs16 = pb_acc.tile([128, 2, W2], F16, tag="rs16")
                nc.vector.tensor_copy(
                    out=rs16.rearrange("p a b -> p (a b)"), in_=rsL)
                with tc.tile_pool(name="pc_o", bufs=3) as pc_o:
                    for c in range(NCH):
                        ob = pc_o.tile([128, 2, W2], F16, tag="ob")
                        for mc in range(2):
                            tt = pc_o.tile([128, W2], F16, tag="tt")
                            nc.gpsimd.tensor_sub(tt, r_all[:, c, mc, :],
                                                 mu16[:, mc, :])
                            nc.vector.tensor_mul(tt, tt, rs16[:, mc, :])
                            nc.vector.tensor_scalar(
                                out=ob[:, mc, :], in0=tt,
                                scalar1=c_gamma[:, c:c + 1],
                                scalar2=c_beta[:, c:c + 1],
                                op0=OP.mult, op1=OP.add)
                        nc.sync.dma_start(
                            out[c].rearrange("(mc p) w -> p mc w", p=128), ob)
    nc.compile()
    return nc


_PROGRAM = None


def kernel(_trace=False, **inputs):
    global _PROGRAM
    np_in = {k: np.ascontiguousarray(np.asarray(v)) for k, v in inputs.items()}
    g, x = np_in["g"], np_in["x"]
    consts = build_consts()

    def pack_gx(a):
        p = np.zeros((C, H, W2), E4M3)
        p[:, :, :W] = a.astype(E4M3)
        return p

    def pack_w(wc, sl):
        return np.ascontiguousarray((wc[sl].T * CS).astype(E4M3))

    def pack_filt(f):
        # f [32, H, WF, 2] -> [16, 4(c,ri), H, WF] fp16, pre-divided by CS
        # to undo the x16 fp8 scaling of the dht constants.
        m = np.moveaxis(f, 3, 1) * (1.0 / CS)  # [32, 2, H, WF]
        return np.ascontiguousarray(
            m.reshape(16, 4, H, WF).astype(np.float16))

    in_maps = []
    for k in range(N_CORES):
        b, grp = k // 4, k % 4
        sl = slice(grp * NCH, (grp + 1) * NCH)
        xr = np.zeros((NCH, H, W2), np.float16)
        xr[:, :, :W] = x[b][sl].astype(np.float16)
        xr[:, 0, :W] += _corr_w().astype(np.float16)
        m = dict(
            g8=pack_gx(g[b]),
            x8=pack_gx(x[b]),
            wgT8=pack_w(np_in["wg_conv"], sl),
            wxT8=pack_w(np_in["wx_conv"], sl),
            bg=np.ascontiguousarray(
                np.tile(np_in["bg_conv"][sl], 4)[:, None]).astype(np.float32),
            bx=np.ascontiguousarray(
                np.tile(np_in["bx_conv"][sl], 4)[:, None]).astype(np.float32),
            fpg=pack_filt(np_in["filt_g"][sl]),
            fpx=pack_filt(np_in["filt_x"][sl]),
            xres=xr,
            gamma=np.ascontiguousarray(
                np_in["ln_gamma"][sl][None, :]).astype(np.float32),
            beta=np.ascontiguousarray(
                np_in["ln_beta"][sl][None, :]).astype(np.float32),
            **consts,
        )
        in_maps.append(m)
    if _PROGRAM is None:
        _PROGRAM = build_program()
    res = run_bass_kernel_spmd(_PROGRAM, in_maps, core_ids=list(range(N_CORES)),
                               trace=_trace)
    outf = np.zeros((B, C, H, W), np.float32)
    for k in range(N_CORES):
        b, grp = k // 4, k % 4
        outf[b, grp * NCH:(grp + 1) * NCH] = \
            res.results[k]["out"][:, :, :W].astype(np.float32)
    if _trace:
        kernel.last_results = res
    return outf


if __name__ == "__main__":
    ins = {
        "g": np.random.randn(B, C, H, W).astype(np.float32),
        "x": np.random.randn(B, C, H, W).astype(np.float32),
        "wg_conv": (np.random.randn(C, C) * 0.05).astype(np.float32),
        "bg_conv": np.zeros(C, np.float32),
        "wx_conv": (np.random.randn(C, C) * 0.05).astype(np.float32),
        "bx_conv": np.zeros(C, np.float32),
        "filt_g": (np.random.randn(C, H, WF, 2) * 0.02).astype(np.float32),
        "filt_x": (np.random.randn(C, H, WF, 2) * 0.02).astype(np.float32),
        "ln_gamma": np.ones(C, np.float32),
        "ln_beta": np.zeros(C, np.float32),
    }
    o = kernel(**ins)
    print("kernel ran, out shape", o.shape)


# revision 25
# speedup vs baseline: 1.0193x; 1.0029x over previous
"""Trainium2 Bass kernel for nn_AttentionFilter (B=2,C=128,H=256,W=510).

Sharding: 8 cores = 2 batches x 4 channel-groups of 32. Per core:
  Phase A: 1x1 conv as fp8 matmul (x16-scaled fp8 weights, 1/16+bias fused
    into the PSUM->SBUF copy), spill y as fp8 to DRAM padded to W=512.
  Phase B (software-pipelined, backend of channel c-1 interleaved with
    frontend of channel c): xbar DMA-transpose of fp8 y viewed as uint16
    pairs -> pair-interleaved [w/2, 2, h] stationaries; rfft_w and fft_h as
    fp8 DoubleRow matmuls (x16-scaled fp8 DFT constants, rescale fused into
    PSUM copies); complex filter multiply as fp16 2x-mode tensor_tensor;
    variance via bn_stats + integer-magic rsqrt on Pool; freq attention
    matmul fp16 into a single PSUM bank; tanh on ACT (sigmoid = 0.5 +
    0.5tanh, the 0.5-DC correction folded into xres on host) with fused
    rstd/2 row scale writing fp8; ifft_i and irfft_j as fp8 DoubleRow
    matmuls; residual add fused with 1/16 rescale; LN stats: S1/S2 fp32
    accumulated on Pool, squares via ACT (x1/256 to bound range).
  Phase C: fp32 AllReduce of LN stats within each batch group, rsqrt via
    integer magic + 2 Newton steps, per-channel normalize, fp16 output.
"""
import sys

sys.path.insert(0, "/opt/trn_rl_repo")

import numpy as np
import ml_dtypes

import concourse.bass as bass
import concourse.mybir as mybir
import concourse.tile as tile
from concourse import bacc
from concourse.bass_utils import run_bass_kernel_spmd

B, C, H, W = 2, 128, 256, 510
WF = 256
W2 = 512
NCH = 32  # channels per core
N_CORES = 8
F32 = mybir.dt.float32
F16 = mybir.dt.float16
F8 = mybir.dt.float8e4
U16 = mybir.dt.uint16
I32 = mybir.dt.int32
AF = mybir.ActivationFunctionType
OP = mybir.AluOpType
DR = mybir.MatmulPerfMode.DoubleRow

E4M3 = ml_dtypes.float8_e4m3
CS = 16.0  # fp8 constant scale
RSQRT_MAGIC = 0x5F3759DF
INV_SQRT_2PI = float(1.0 / np.sqrt(2.0 * np.pi))

_CORR = None


def _corr_w():
    # irfft2 of the constant 0.5 field of atten (sigmoid = 0.5 + 0.5tanh):
    # after ifft over i it is 8*(1+1j) at m=0; irfft over j gives this
    # w-profile on the h=0 row.
    global _CORR
    if _CORR is None:
        AR = np.fft.irfft(np.eye(WF), n=W, axis=0, norm="ortho")
        AI = np.fft.irfft(1j * np.eye(WF), n=W, axis=0, norm="ortho")
        _CORR = 8.0 * (AR.sum(axis=1) + AI.sum(axis=1))
    return _CORR


def build_consts():
    Fw = np.fft.rfft(np.eye(W), axis=0, norm="ortho").T  # [W, WF] complex
    fw_pack = np.zeros((W2, W2), np.float32)
    fw_pack[:W, :WF] = Fw.real
    fw_pack[:W, WF:] = Fw.imag
    # pair-interleaved for DoubleRow: fw8[k, j, n] = fw_pack[2k+j, n] * CS
    fw8 = (fw_pack * CS).reshape(WF, 2, W2)
    DH = np.fft.fft(np.eye(H), axis=0, norm="ortho")  # [kh, h]
    dht_r = np.ascontiguousarray(DH.real.T * CS).astype(E4M3)  # [h, kh]
    dht_i = np.ascontiguousarray(DH.imag.T * CS).astype(E4M3)
    dht_ni = np.ascontiguousarray((-DH.imag).T * CS).astype(E4M3)
    IDH = np.fft.ifft(np.eye(H), axis=0, norm="ortho")  # [m, i]
    ida = np.zeros((H, W2), np.float32)
    ida[:, :WF] = IDH.real.T * (CS * 0.5)
    ida[:, WF:] = IDH.imag.T * (CS * 0.5)
    idb = np.zeros((H, W2), np.float32)
    idb[:, :WF] = -IDH.imag.T * (CS * 0.5)
    idb[:, WF:] = IDH.real.T * (CS * 0.5)
    AR = np.fft.irfft(np.eye(WF), n=W, axis=0, norm="ortho")  # [w, j]
    AI = np.fft.irfft(1j * np.eye(WF), n=W, axis=0, norm="ortho")
    awr = np.zeros((WF, W2), np.float32)
    awr[:, :W] = AR.T * CS
    awi = np.zeros((WF, W2), np.float32)
    awi[:, :W] = AI.T * CS
    return dict(
        fw8=fw8.astype(E4M3),
        dht_r=dht_r, dht_i=dht_i, dht_ni=dht_ni,
        idht_a=ida.astype(E4M3), idht_b=idb.astype(E4M3),
        awr8=awr.astype(E4M3), awi8=awi.astype(E4M3),
    )


def build_program():
    nc = bacc.Bacc("TRN2", target_bir_lowering=False, debug=False,
                   num_devices=N_CORES)

    def inp(name, shape, dt=F32):
        return nc.dram_tensor(name, list(shape), dt, kind="ExternalInput").ap()

    g8 = inp("g8", (C, H, W2), F8)
    x8 = inp("x8", (C, H, W2), F8)
    wgT8 = inp("wgT8", (C, NCH), F8)
    wxT8 = inp("wxT8", (C, NCH), F8)
    bg = inp("bg", (128, 1))
    bx = inp("bx", (128, 1))
    fw8 = inp("fw8", (WF, 2, W2), F8)
    dht_r = inp("dht_r", (H, H), F8)
    dht_i = inp("dht_i", (H, H), F8)
    dht_ni = inp("dht_ni", (H, H), F8)
    idht_a = inp("idht_a", (H, W2), F8)
    idht_b = inp("idht_b", (H, W2), F8)
    awr8 = inp("awr8", (WF, W2), F8)
    awi8 = inp("awi8", (WF, W2), F8)
    # filters packed per channel-pair: [16, 4(c,ri), 256kh, 256kw], x(1/CS)
    fpg = inp("fpg", (NCH // 2, 4, H, WF), F16)
    fpx = inp("fpx", (NCH // 2, 4, H, WF), F16)
    xres = inp("xres", (NCH, H, W2), F16)
    gamma = inp("gamma", (1, NCH))
    beta = inp("beta", (1, NCH))
    out = nc.dram_tensor("out", [NCH, H, W2], F16, kind="ExternalOutput").ap()

    with tile.TileContext(nc) as tc:
        with (
            tc.tile_pool(name="consts", bufs=1) as consts,
            tc.tile_pool(name="dram", bufs=1, space="DRAM") as dram,
        ):
            # ---- constants into SBUF
            c_fw = consts.tile([128, 2, 2, W2], F8, tag="c_fw")
            nc.sync.dma_start(
                c_fw, fw8.rearrange("(kc p) j n -> p kc j n", p=128))

            def ld2(src, ncol=H):
                t = consts.tile([128, 2, ncol], F8, tag=f"c_{src.name}")
                nc.sync.dma_start(t, src.rearrange("(hc p) m -> p hc m", p=128))
                return t

            c_dhtr = ld2(dht_r)
            c_dhti = ld2(dht_i)
            c_dhtni = ld2(dht_ni)
            c_ida = ld2(idht_a, W2)
            c_idb = ld2(idht_b, W2)
            c_awr = ld2(awr8, W2)
            c_awi = ld2(awi8, W2)
            c_wgT = consts.tile([C, NCH], F8, tag="c_wgT")
            nc.sync.dma_start(c_wgT, wgT8)
            c_wxT = consts.tile([C, NCH], F8, tag="c_wxT")
            nc.sync.dma_start(c_wxT, wxT8)
            c_bg = consts.tile([128, 1], F32, tag="c_bg")
            nc.sync.dma_start(c_bg, bg)
            c_bx = consts.tile([128, 1], F32, tag="c_bx")
            nc.sync.dma_start(c_bx, bx)
            c_gamma = consts.tile([128, NCH], F32, tag="c_gamma")
            nc.sync.dma_start(c_gamma, gamma.to_broadcast([128, NCH]))
            c_beta = consts.tile([128, NCH], F32, tag="c_beta")
            nc.sync.dma_start(c_beta, beta.to_broadcast([128, NCH]))

            # ---- DRAM scratch (y spill stored as uint16 fp8-pairs)
            ysp_g = dram.tile([NCH, H, WF], U16, tag="ysp_g")
            ysp_x = dram.tile([NCH, H, WF], U16, tag="ysp_x")
            cc_in = dram.tile([128, 2048], F32, tag="cc_in")
            cc_out = dram.tile([128, 2048], F32, tag="cc_out")

            # ---- Phase A: 1x1 conv (fp8), spill y fp8
            HB = 32
            with (
                tc.tile_pool(name="pa_in", bufs=3) as pa_in,
                tc.tile_pool(name="pa_out", bufs=3) as pa_out,
                tc.tile_pool(name="pa_ps", bufs=4, space="PSUM") as pa_ps,
            ):
                for srct, wTt, biast, yspt in (
                    (g8, c_wgT, c_bg, ysp_g),
                    (x8, c_wxT, c_bx, ysp_x),
                ):
                    for blk in range(H // HB):
                        h0 = blk * HB
                        rh = pa_in.tile([C, HB, W2], F8, tag="rh")
                        nc.sync.dma_start(rh, srct[:, h0:h0 + HB, :])
                        stag = pa_out.tile([128, HB // 4, W2], F8, tag="stag")
                        for i2 in range(HB // 4):
                            ps = pa_ps.tile([128, W2], F32, tag="cps")
                            for j in range(4):
                                nc.tensor.matmul(
                                    ps[32 * j:32 * (j + 1), :], wTt,
                                    rh[:, i2 * 4 + j, :],
                                    start=True, stop=True,
                                    tile_position=(0, 32 * j))
                            if i2 % 2 == 0:
                                nc.scalar.activation(
                                    stag[:, i2, :], ps, AF.Identity,
                                    bias=biast, scale=1.0 / CS)
                            else:
                                nc.vector.tensor_scalar(
                                    out=stag[:, i2, :], in0=ps,
                                    scalar1=1.0 / CS, scalar2=biast,
                                    op0=OP.mult, op1=OP.add)
                        stag16 = stag.bitcast(U16)
                        for j in range(4):
                            nc.sync.dma_start(
                                yspt[:, h0 + j:h0 + HB:4, :],
                                stag16[32 * j:32 * (j + 1), :, :])

            # ---- Phase B: software-pipelined per-channel pipeline
            with (
                tc.tile_pool(name="pb_yt", bufs=2) as pb_yt,
                tc.tile_pool(name="pb_yw", bufs=4) as pb_yw,
                tc.tile_pool(name="pb_zp", bufs=4) as pb_zp,
                tc.tile_pool(name="pb_z", bufs=3) as pb_z,
                tc.tile_pool(name="pb_f", bufs=2) as pb_f,
                tc.tile_pool(name="pb_sm", bufs=3) as pb_sm,
                tc.tile_pool(name="pb_att", bufs=3) as pb_att,
                tc.tile_pool(name="pb_inv", bufs=3) as pb_inv,
                tc.tile_pool(name="pb_x", bufs=2) as pb_x,
                tc.tile_pool(name="pb_acc", bufs=1) as pb_acc,
                tc.tile_pool(name="pb_pw", bufs=2, space="PSUM") as pb_pw,
                tc.tile_pool(name="pb_pf", bufs=2, space="PSUM") as pb_pf,
                tc.tile_pool(name="pb_pt", bufs=4, space="PSUM") as pb_pt,
            ):
                S1 = pb_acc.tile([128, 2, W2], F32, tag="S1")
                S2 = pb_acc.tile([128, 2, W2], F32, tag="S2")
                nc.vector.memset(S1, 0.0)
                nc.vector.memset(S2, 0.0)
                r_all = pb_acc.tile([128, NCH, 2, W2], F16, tag="r_all")

                ytT = {}
                fP = {}
                xc_d = {}

                def frontend(c):
                    cc = c % 2
                    cc4 = c % 4
                    if cc4 == 0:
                        # xbar-transpose 4 channels at a time
                        for t, ysp in ((0, ysp_g), (1, ysp_x)):
                            yt = pb_yt.tile([128, 2, 4, WF], U16, tag=f"yt{t}")
                            for kc in range(2):
                                nc.sync.dma_start_transpose(
                                    yt[:, kc, :, :].rearrange(
                                        "p c h -> p (c h)"),
                                    ysp[c:c + 4, :,
                                        kc * 128:(kc + 1) * 128].rearrange(
                                        "c h w -> (c h) w"))
                            ytT[t] = yt
                    if cc == 0:
                        for t, fpd in ((0, fpg), (1, fpx)):
                            fpt = pb_f.tile([128, 4, 2, WF], F16, tag=f"fp{t}")
                            nc.sync.dma_start(
                                fpt, fpd[c // 2].rearrange(
                                    "cr (khc p) k -> p cr khc k", p=128))
                            fP[t] = fpt
                        xc = pb_x.tile([128, 2, 2, W2], F16, tag="xc")
                        nc.sync.dma_start(
                            xc, xres[c:c + 2].rearrange(
                                "c (mc p) w -> p c mc w", p=128))
                        xc_d[c // 2] = xc
                    z = {}
                    for t in (0, 1):
                        yt8 = ytT[t].bitcast(F8)  # [128, 2kc, 4c, 512]
                        # B1: rfft_w as fp8 DoubleRow over w-pairs
                        yw8 = pb_yw.tile([128, 2, W2], F8, tag="yw8")
                        for hc in range(2):
                            pw = pb_pw.tile([128, W2], F32, tag="pw")
                            first = True
                            for kc in range(2):
                                lhsv = yt8[:, kc, cc4, :].rearrange(
                                    "p (h j) -> p j h", j=2)
                                for j in range(2):
                                    nc.tensor.matmul(
                                        pw,
                                        lhsv[:, j,
                                             hc * 128:(hc + 1) * 128],
                                        c_fw[:, kc, j, :],
                                        start=first,
                                        stop=(kc == 1 and j == 1))
                                    first = False
                            nc.scalar.activation(
                                yw8[:, hc, :], pw, AF.Identity,
                                scale=1.0 / CS)
                        # B2: fft_h as fp8 DoubleRow, R and I into one bank
                        ywR = yw8[:, :, 0:WF]
                        ywI = yw8[:, :, WF:W2]
                        zP = pb_zp.tile([128, 2, W2], F16, tag="zP")
                        for khc in range(2):
                            pf = pb_pf.tile([128, W2], F32, tag="pf")
                            ksl = slice(khc * 128, (khc + 1) * 128)
                            nc.tensor.matmul(
                                pf[:, 0:WF], c_dhtr[:, :, ksl], ywR,
                                start=True, stop=False, perf_mode=DR,
                                skip_group_check=True)
                            nc.tensor.matmul(
                                pf[:, 0:WF], c_dhtni[:, :, ksl], ywI,
                                start=False, stop=False, perf_mode=DR,
                                skip_group_check=True)
                            nc.tensor.matmul(
                                pf[:, WF:W2], c_dhti[:, :, ksl], ywR,
                                start=False, stop=False, perf_mode=DR,
                                skip_group_check=True)
                            nc.tensor.matmul(
                                pf[:, WF:W2], c_dhtr[:, :, ksl], ywI,
                                start=False, stop=True, perf_mode=DR,
                                skip_group_check=True)
                            nc.scalar.copy(out=zP[:, khc, :], in_=pf)
                        # B3: filter multiply (fp16 2x TT ops)
                        zRp = zP[:, :, 0:WF]
                        zIp = zP[:, :, WF:W2]
                        fR = fP[t][:, 2 * cc, :, :]
                        fI = fP[t][:, 2 * cc + 1, :, :]
                        t1 = pb_sm.tile([128, 2, WF], F16, tag="t1")
                        t2 = pb_sm.tile([128, 2, WF], F16, tag="t2")
                        t3 = pb_sm.tile([128, 2, WF], F16, tag="t3")
                        t4 = pb_sm.tile([128, 2, WF], F16, tag="t4")
                        nc.vector.tensor_mul(t1, zRp, fR)
                        nc.vector.tensor_mul(t2, zIp, fI)
                        nc.vector.tensor_mul(t3, zRp, fI)
                        nc.vector.tensor_mul(t4, zIp, fR)
                        zR = pb_z.tile([128, 2, WF], F16, tag=f"zR{t}")
                        zI = pb_z.tile([128, 2, WF], F16, tag=f"zI{t}")
                        nc.vector.tensor_sub(zR, t1, t2)
                        nc.vector.tensor_add(zI, t3, t4)
                        z[(t, "R")] = zR
                        z[(t, "I")] = zI
                        if t == 1:
                            nzI = pb_z.tile([128, 2, WF], F16, tag="nzI")
                            nc.vector.tensor_scalar_mul(nzI, zI, -1.0)
                            z[(1, "nI")] = nzI
                    # B4: variance over kw per kh row -> rstd (Pool)
                    gR, gI = z[(0, "R")], z[(0, "I")]
                    v2 = pb_sm.tile([128, 2], F32, tag="v2")
                    for khc in range(2):
                        st = pb_sm.tile([128, 2, 6], F32, tag="bst")
                        nc.vector.bn_stats(out=st[:, 0, :], in_=gR[:, khc, :])
                        nc.vector.bn_stats(out=st[:, 1, :], in_=gI[:, khc, :])
                        mvR = pb_sm.tile([128, 2], F32, tag="mvR")
                        mvI = pb_sm.tile([128, 2], F32, tag="mvI")
                        nc.vector.bn_aggr(out=mvR, in_=st[:, 0, :])
                        nc.vector.bn_aggr(out=mvI, in_=st[:, 1, :])
                        nc.gpsimd.tensor_add(v2[:, khc:khc + 1],
                                             mvR[:, 1:2], mvI[:, 1:2])
                    ti = pb_sm.tile([128, 2], I32, tag="ti")
                    nc.vector.tensor_scalar(
                        out=ti, in0=v2.bitcast(I32), scalar1=1, scalar2=0,
                        op0=OP.arith_shift_right, op1=OP.bypass)
                    nc.vector.tensor_scalar(
                        out=ti, in0=ti, scalar1=-1, scalar2=RSQRT_MAGIC,
                        op0=OP.mult, op1=OP.add)
                    y0 = ti.bitcast(F32)
                    tn = pb_sm.tile([128, 2], F32, tag="tn")
                    nc.gpsimd.tensor_mul(tn, y0, y0)
                    nc.gpsimd.tensor_mul(tn, tn, v2)
                    # fold the 0.5/sqrt(2pi) tanh prescale into the Newton
                    # constants: rstd = y0*(1.5 - 0.5*v*y0^2)*K = y0*tn'
                    K = INV_SQRT_2PI * 0.5
                    nc.vector.tensor_scalar(
                        out=tn, in0=tn, scalar1=-0.5 * K, scalar2=1.5 * K,
                        op0=OP.mult, op1=OP.add)
                    rstd = pb_sm.tile([128, 2], F32, tag="rstd")
                    nc.gpsimd.tensor_mul(rstd, y0, tn)
                    return z, rstd

                def backend1(c, z, rstd):
                    gR, gI = z[(0, "R")], z[(0, "I")]
                    xR, xI, nxI = z[(1, "R")], z[(1, "I")], z[(1, "nI")]
                    # B5: scores (fp16) into one PSUM bank + tanh -> fp8
                    a8 = pb_att.tile([128, 2, W2], F8, tag="a8")
                    for ic in range(2):
                        psc = pb_pt.tile([128, W2], F32, tag="pt")
                        isl = slice(ic * 128, (ic + 1) * 128)
                        for khc in range(2):
                            nc.tensor.matmul(
                                psc[:, 0:WF], gR[:, khc, isl], xR[:, khc, :],
                                start=(khc == 0), stop=False,
                                skip_group_check=True)
                            nc.tensor.matmul(
                                psc[:, 0:WF], gI[:, khc, isl], nxI[:, khc, :],
                                start=False, stop=False,
                                skip_group_check=True)
                            nc.tensor.matmul(
                                psc[:, WF:W2], gR[:, khc, isl], xI[:, khc, :],
                                start=False, stop=False,
                                skip_group_check=True)
                            nc.tensor.matmul(
                                psc[:, WF:W2], gI[:, khc, isl], xR[:, khc, :],
                                start=False, stop=(khc == 1),
                                skip_group_check=True)
                        nc.scalar.activation(a8[:, ic, :], psc, AF.Tanh,
                                             scale=rstd[:, ic:ic + 1])
                    return a8

                def backend2(c, a8):
                    cc = c % 2
                    xc = xc_d[c // 2]
                    # B7: ifft over i (fp8 DoubleRow), inv = pv/16
                    inv8 = pb_inv.tile([128, 2, W2], F8, tag="inv8")
                    for jc in range(2):
                        pv = pb_pt.tile([128, W2], F32, tag="pt")
                        jsl = slice(jc * 128, (jc + 1) * 128)
                        jsl2 = slice(WF + jc * 128, WF + (jc + 1) * 128)
                        nc.tensor.matmul(pv, a8[:, :, jsl], c_ida,
                                         start=True, stop=False, perf_mode=DR)
                        nc.tensor.matmul(pv, a8[:, :, jsl2], c_idb,
                                         start=False, stop=True, perf_mode=DR)
                        nc.scalar.activation(inv8[:, jc, :], pv, AF.Identity,
                                             scale=1.0 / CS)
                    # B8: irfft over j (fp8 DoubleRow) + residual + stats
                    for mc in range(2):
                        pr = pb_pt.tile([128, W2], F32, tag="pt")
                        msl = slice(mc * 128, (mc + 1) * 128)
                        msl2 = slice(WF + mc * 128, WF + (mc + 1) * 128)
                        nc.tensor.matmul(pr, inv8[:, :, msl], c_awr,
                                         start=True, stop=False, perf_mode=DR)
                        nc.tensor.matmul(pr, inv8[:, :, msl2], c_awi,
                                         start=False, stop=True, perf_mode=DR)
                        rc = r_all[:, c, mc, :]
                        nc.vector.scalar_tensor_tensor(
                            out=rc, in0=pr, scalar=1.0 / CS,
                            in1=xc[:, cc, mc, :], op0=OP.mult, op1=OP.add)
                        nc.gpsimd.tensor_add(S1[:, mc, :], S1[:, mc, :], rc)
                        sq = pb_sm.tile([128, W2], F16, tag="sq")
                        nc.scalar.activation(sq, rc, AF.Square,
                                             scale=1.0 / 16.0)
                        nc.gpsimd.tensor_add(S2[:, mc, :], S2[:, mc, :], sq)

                state = {}
                att_st = {}
                for c in range(NCH + 2):
                    if c < NCH:
                        state[c] = frontend(c)
                    if 1 <= c <= NCH:
                        att_st[c - 1] = backend1(c - 1, *state.pop(c - 1))
                    if c >= 2:
                        backend2(c - 2, att_st.pop(c - 2))

                # ---- Phase C: LN stats AllReduce + normalize
                nc.sync.dma_start(cc_in[:, 0:1024],
                                  S1.rearrange("p a b -> p (a b)"))
                nc.sync.dma_start(cc_in[:, 1024:2048],
                                  S2.rearrange("p a b -> p (a b)"))
                nc.gpsimd.collective_compute(
                    "AllReduce", OP.add,
                    replica_groups=[[0, 1, 2, 3], [4, 5, 6, 7]],
                    ins=[cc_in.opt()], outs=[cc_out.opt()])
                mu32 = pb_acc.tile([128, 1024], F32, tag="mu32")
                nc.sync.dma_start(mu32, cc_out[:, 0:1024])
                nc.vector.tensor_scalar_mul(mu32, mu32, 1.0 / C)
                e2 = pb_acc.tile([128, 1024], F32, tag="e2")
                nc.sync.dma_start(e2, cc_out[:, 1024:2048])
                nc.vector.tensor_scalar_mul(e2, e2, 256.0 / C)
                var = pb_acc.tile([128, 1024], F32, tag="var")
                nc.vector.scalar_tensor_tensor(
                    out=var, in0=mu32, scalar=-1.0, in1=mu32,
                    op0=OP.mult, op1=OP.mult)
                nc.vector.scalar_tensor_tensor(
                    out=var, in0=var, scalar=1e-6, in1=e2,
                    op0=OP.add, op1=OP.add)
                tiL = pb_acc.tile([128, 1024], I32, tag="tiL")
                nc.vector.tensor_scalar(
                    out=tiL, in0=var.bitcast(I32), scalar1=1, scalar2=0,
                    op0=OP.arith_shift_right, op1=OP.bypass)
                nc.vector.tensor_scalar(
                    out=tiL, in0=tiL, scalar1=-1, scalar2=RSQRT_MAGIC,
                    op0=OP.mult, op1=OP.add)
                y0L = tiL.bitcast(F32)
                tnL = pb_acc.tile([128, 1024], F32, tag="tnL")
                rsL = pb_acc.tile([128, 1024], F32, tag="rsL")
                for it in range(2):
                    nc.vector.scalar_tensor_tensor(
                        out=tnL, in0=y0L, scalar=1.0, in1=y0L,
                        op0=OP.bypass, op1=OP.mult)
                    nc.vector.scalar_tensor_tensor(
                        out=tnL, in0=tnL, scalar=1.0, in1=var,
                        op0=OP.bypass, op1=OP.mult)
                    nc.vector.tensor_scalar(
                        out=tnL, in0=tnL, scalar1=-0.5, scalar2=1.5,
                        op0=OP.mult, op1=OP.add)
                    nc.vector.scalar_tensor_tensor(
                        out=rsL, in0=y0L, scalar=1.0, in1=tnL,
                        op0=OP.bypass, op1=OP.mult)
                    y0L = rsL
                mu16 = pb_acc.tile([128, 2, W2], F16, tag="mu16")
                nc.vector.tensor_copy(
                    out=mu16.rearrange("p a b -> p (a b)"), in_=mu32)
                rs16 = pb_acc.tile([128, 2, W2], F16, tag="rs16")
                nc.vector.tensor_copy(
                    out=rs16.rearrange("p a b -> p (a b)"), in_=rsL)
                with tc.tile_pool(name="pc_o", bufs=3) as pc_o:
                    for c in range(NCH):
                        ob = pc_o.tile([128, 2, W2], F16, tag="ob")
                        for mc in range(2):
                            tt = pc_o.tile([128, W2], F16, tag="tt")
                            nc.gpsimd.tensor_sub(tt, r_all[:, c, mc, :],
                                                 mu16[:, mc, :])
                            nc.vector.tensor_mul(tt, tt, rs16[:, mc, :])
                            nc.vector.tensor_scalar(
                                out=ob[:, mc, :], in0=tt,
                                scalar1=c_gamma[:, c:c + 1],
                                scalar2=c_beta[:, c:c + 1],
                                op0=OP.mult, op1=OP.add)
                        nc.sync.dma_start(
                            out[c].rearrange("(mc p) w -> p mc w", p=128), ob)
    nc.compile()
    return nc


_PROGRAM = None


def kernel(_trace=False, **inputs):
    global _PROGRAM
    np_in = {k: np.ascontiguousarray(np.asarray(v)) for k, v in inputs.items()}
    g, x = np_in["g"], np_in["x"]
    consts = build_consts()

    def pack_gx(a):
        p = np.zeros((C, H, W2), E4M3)
        p[:, :, :W] = a.astype(E4M3)
        return p

    def pack_w(wc, sl):
        return np.ascontiguousarray((wc[sl].T * CS).astype(E4M3))

    def pack_filt(f):
        # f [32, H, WF, 2] -> [16, 4(c,ri), H, WF] fp16, pre-divided by CS
        # to undo the x16 fp8 scaling of the dht constants.
        m = np.moveaxis(f, 3, 1) * (1.0 / CS)  # [32, 2, H, WF]
        return np.ascontiguousarray(
            m.reshape(16, 4, H, WF).astype(np.float16))

    in_maps = []
    for k in range(N_CORES):
        b, grp = k // 4, k % 4
        sl = slice(grp * NCH, (grp + 1) * NCH)
        xr = np.zeros((NCH, H, W2), np.float16)
        xr[:, :, :W] = x[b][sl].astype(np.float16)
        xr[:, 0, :W] += _corr_w().astype(np.float16)
        m = dict(
            g8=pack_gx(g[b]),
            x8=pack_gx(x[b]),
            wgT8=pack_w(np_in["wg_conv"], sl),
            wxT8=pack_w(np_in["wx_conv"], sl),
            bg=np.ascontiguousarray(
                np.tile(np_in["bg_conv"][sl], 4)[:, None]).astype(np.float32),
            bx=np.ascontiguousarray(
                np.tile(np_in["bx_conv"][sl], 4)[:, None]).astype(np.float32),
            fpg=pack_filt(np_in["filt_g"][sl]),
            fpx=pack_filt(np_in["filt_x"][sl]),
            xres=xr,
            gamma=np.ascontiguousarray(
                np_in["ln_gamma"][sl][None, :]).astype(np.float32),
            beta=np.ascontiguousarray(
                np_in["ln_beta"][sl][None, :]).astype(np.float32),
            **consts,
        )
        in_maps.append(m)
    if _PROGRAM is None:
        _PROGRAM = build_program()
    res = run_bass_kernel_spmd(_PROGRAM, in_maps, core_ids=list(range(N_CORES)),
                               trace=_trace)
    outf = np.zeros((B, C, H, W), np.float32)
    for k in range(N_CORES):
        b, grp = k // 4, k % 4
        outf[b, grp * NCH:(grp + 1) * NCH] = \
            res.results[k]["out"][:, :, :W].astype(np.float32)
    if _trace:
        kernel.last_results = res
    return outf


if __name__ == "__main__":
    ins = {
        "g": np.random.randn(B, C, H, W).astype(np.float32),
        "x": np.random.randn(B, C, H, W).astype(np.float32),
        "wg_conv": (np.random.randn(C, C) * 0.05).astype(np.float32),
        "bg_conv": np.zeros(C, np.float32),
        "wx_conv": (np.random.randn(C, C) * 0.05).astype(np.float32),
        "bx_conv": np.zeros(C, np.float32),
        "filt_g": (np.random.randn(C, H, WF, 2) * 0.02).astype(np.float32),
        "filt_x": (np.random.randn(C, H, WF, 2) * 0.02).astype(np.float32),
        "ln_gamma": np.ones(C, np.float32),
        "ln_beta": np.zeros(C, np.float32),
    }
    o = kernel(**ins)
    print("kernel ran, out shape", o.shape)
